# revision 16
# baseline (speedup 1.0000x reference)
"""Single-head causal attention (B=8, T=2048, C=768, H=64) on 8 TRN2 NeuronCores.

Sharding: data-parallel over the batch dim — one batch element per core.

v4 schedule — input DMA, the softmax exp stream (ScalarE), and the PE matmul
stream are the three pacers; every queue is kept free of FIFO hazards:

  - x^T bf16 as TWO big descriptors (low column half, then high) on the sync
    HWDGE ring ONLY — SDMA engines round-robin rings at packet granularity,
    so any transfer queued on another ring would steal bandwidth from the
    low half that unblocks the first projections.
  - consts + the kT partition-shift SBUF->SBUF DMAs ride the otherwise-empty
    scalar HWDGE ring; gpsimd does memsets + the causal tri-mask multiplies.
  - ~12 warmup matmuls hold the PE HAM activity monitor until data lands.
  - qk projections (fused [Wq|Wk] stationary) first; score strips
    S^T [128, <=1024] + exp (scale fused) start as soon as kT cols 0:512
    exist; v projections + PE-transposes are deferred INTO the AV phase,
    filling the PE while it would otherwise wait on the exp stream.
  - S strips for group 1 are emitted exp-paced (+2 ahead of their AV) so the
    in-order PE queue never head-blocks on a PSUM slot.
  - AV: out^T [65, 1024] += [v_j | 1].T @ expS^T_j; finalize per 128-col
    t-chunk as soon as its last AV lands (copy [65,128] -> bf16 SBUF,
    PE-transpose to [128,65], reciprocal of col 64, tensor_scalar mul into a
    staging tile); 4 output DMAs on sync, the last covering one t-chunk.

No max-subtraction in softmax: scores * C**-0.5 are bounded (|s| < ~3), exp is
safe in fp32, and the result is mathematically identical to jax.nn.softmax.
"""

import ml_dtypes
import numpy as np

import concourse.bass as bass
import concourse.tile as tile
from concourse import bacc, mybir
from concourse.bass import ds, ts

B, T, C, H = 8, 2048, 768, 64
P = 128
NCH = C // P          # 6 contraction chunks for QKV
GW = 1024             # attention output column-group width
NG = T // GW          # 2 groups
NT = T // P           # 16 t-chunks
JPG = GW // P         # 8 j-chunks per group
SCALE = float(C) ** -0.5
N_WARMUP = 12

F32 = mybir.dt.float32
BF16 = mybir.dt.bfloat16
EXP = mybir.ActivationFunctionType.Exp


def _emit(tc: tile.TileContext, ctx, xlo, xhi, wqk, wv, ident, tri, out):
    nc = tc.nc

    consts = ctx.enter_context(tc.tile_pool(name="consts", bufs=1))
    xpool = ctx.enter_context(tc.tile_pool(name="x", bufs=1))
    qpool = ctx.enter_context(tc.tile_pool(name="qkv", bufs=1))
    prb0p = ctx.enter_context(tc.tile_pool(name="prb0", bufs=8))
    prb1p = ctx.enter_context(tc.tile_pool(name="prb1", bufs=16))
    fin = ctx.enter_context(tc.tile_pool(name="fin", bufs=3))
    osbp = ctx.enter_context(tc.tile_pool(name="osb", bufs=1))

    qkT = qpool.tile([P, T], BF16)   # rows 0:64 q^T, 64:128 k^T
    kT = qpool.tile([H, T], BF16)    # k^T shifted to base partition 0
    vT = qpool.tile([H, T], BF16)
    v_sb = qpool.tile([P, NT, H + 1], BF16)
    o_sb = osbp.tile([P, NT, H], F32)
    dum = qpool.tile([P, 512], BF16)

    # gpsimd: memsets first (unblock warmup); its SWDGE ring carries only
    # the small kT partition-shift SBUF->SBUF DMAs
    nc.gpsimd.memset(dum[:], 0.0)
    nc.gpsimd.memset(v_sb[:, :, H : H + 1], 1.0)

    # x split across BOTH HWDGE rings (each ring caps out well below the HBM
    # per-core limit), low column half strictly first on each ring. ALL small
    # consts go before the big x pieces: a const queued behind a 768KB
    # transfer stalls whichever engine queue the scheduler hoists its wait to.
    xT_sb = xpool.tile([P, NCH, T], BF16)
    w_qk = consts.tile([P, NCH, P], BF16)
    tri_sb = consts.tile([P, P], BF16)
    w_v = consts.tile([P, NCH, H], BF16)
    ident_sb = consts.tile([P, P], BF16)
    # sync ring
    nc.sync.dma_start(w_qk[:], wqk.rearrange("(o p) m -> p o m", p=P))
    nc.sync.dma_start(tri_sb[:], tri)
    nc.sync.dma_start(xT_sb[:, 0:3, 0:1024], xlo[:, 0:3, :])
    nc.sync.dma_start(xT_sb[:, 0:3, 1024:2048], xhi[:, 0:3, :])
    # scalar ring
    nc.scalar.dma_start(w_v[:], wv.rearrange("(o p) m -> p o m", p=P))
    nc.scalar.dma_start(ident_sb[:], ident)
    nc.scalar.dma_start(xT_sb[:, 3:6, 0:1024], xlo[:, 3:6, :])
    nc.scalar.dma_start(xT_sb[:, 3:6, 1024:2048], xhi[:, 3:6, :])

    # S^T strips PSUM (2 x 2 banks), lives through both phases
    sp = ctx.enter_context(tc.tile_pool(name="spsum", bufs=2, space="PSUM"))

    prbs = {}

    def emit_S(g, jj):
        istart = max(g * GW, jj * P)
        n = (g + 1) * GW - istart
        sps = sp.tile([P, GW], F32, tag="s", name=f"s_{g}_{jj}")
        for h in range(0, n, 512):
            nh = min(512, n - h)
            nc.tensor.matmul(
                sps[:, h : h + nh],
                kT[:, ts(jj, P)],
                qkT[0:H, ds(istart + h, nh)],
                start=True,
                stop=True,
            )
        pool, tag = (prb0p, "p0") if g == 0 else (prb1p, "p1")
        prb = pool.tile([P, GW], BF16, tag=tag, name=f"prb_{g}_{jj}")
        nc.scalar.activation(prb[:, :n], sps[:, :n], EXP, scale=SCALE)
        if jj >= JPG * g:
            # leading 128 cols are the diagonal block: upper-tri (j<=i) mask
            nc.gpsimd.tensor_mul(out=prb[:, :P], in0=prb[:, :P], in1=tri_sb[:])
        prbs[(g, jj)] = prb

    def emit_AV(g, jj, ops):
        istart = max(g * GW, jj * P)
        n = (g + 1) * GW - istart
        ioff = istart - g * GW
        prb = prbs[(g, jj)]
        # split at the ops tile's absolute 512-col PSUM bank boundaries
        seg = ioff
        while seg < ioff + n:
            seg_end = min(ioff + n, (seg // 512 + 1) * 512)
            half = seg // 512
            # last j-chunk writing this 512-wide half of the group
            jj_last = min(JPG * g + JPG - 1, JPG * g + 4 * (half + 1) - 1)
            nc.tensor.matmul(
                ops[:, seg:seg_end],
                v_sb[:, jj, :],
                prb[:, seg - ioff : seg_end - ioff],
                start=(jj == 0),
                stop=(jj == jj_last),
            )
            seg = seg_end

    # ---- Phase A: warmup + qk projections + group-0 score strips ----
    with tc.tile_pool(name="pqk", bufs=3, space="PSUM") as pqk:
        for w in range(N_WARMUP):
            dps = pqk.tile([P, 512], F32, tag="p", name=f"warm_{w}")
            nc.tensor.matmul(dps[:], dum[:, 0:P], dum[:], start=True, stop=True)

        def emit_qk(g4):
            ps = pqk.tile([P, 512], F32, tag="p", name=f"qk_{g4}")
            for c in range(NCH):
                nc.tensor.matmul(
                    ps[:],
                    w_qk[:, c, :],
                    xT_sb[:, c, ts(g4, 512)],
                    start=(c == 0),
                    stop=(c == NCH - 1),
                )
            nc.vector.tensor_copy(qkT[:, ts(g4, 512)], ps[:])
            # k^T lives at partitions 64:128; shift to base partition 0 on
            # the gpsimd SWDGE ring (empty: never behind a big HBM transfer)
            nc.gpsimd.dma_start(kT[:, ts(g4, 512)], qkT[H:P, ts(g4, 512)])

        emit_qk(0)
        emit_qk(1)
        for jj in range(4):
            emit_S(0, jj)
        emit_qk(2)
        emit_S(0, 4)
        emit_S(0, 5)
        emit_qk(3)
        emit_S(0, 6)
        emit_S(0, 7)

    # ---- Phase B: v projections + AV + per-t-chunk finalize ----
    # tps: shared 2-slot PSUM pool for v-projection psum, v transposes, and
    # finalize transposes (each <= 1 bank)
    op = ctx.enter_context(tc.tile_pool(name="opsum", bufs=1, space="PSUM"))
    tps = ctx.enter_context(tc.tile_pool(name="tps", bufs=2, space="PSUM"))

    def emit_v(g4):
        ps = tps.tile([H, 512], F32, tag="t", name=f"v_{g4}")
        for c in range(NCH):
            nc.tensor.matmul(
                ps[:],
                w_v[:, c, :],
                xT_sb[:, c, ts(g4, 512)],
                start=(c == 0),
                stop=(c == NCH - 1),
            )
        nc.vector.tensor_copy(vT[:, ts(g4, 512)], ps[:])

    def emit_vtrans(t):
        pt = tps.tile([P, H], BF16, tag="t", name=f"vt_{t}")
        nc.tensor.transpose(pt[:], vT[:, ts(t, P)], ident_sb[0:H, 0:H])
        nc.vector.tensor_copy(v_sb[:, t, 0:H], pt[:])

    def emit_fin(g, c, ops):
        # t-chunk tt = 8g + c is complete once AV(g, JPG*g + c) has landed
        tt = g * JPG + c
        ot = fin.tile([H + 1, P], BF16, tag="ot", name=f"ot_{tt}")
        if g == 1 and c >= 3:
            # tail chunks: exp stream is finished, ScalarE is free — take the
            # PSUM->SBUF copy off VectorE so the tail isn't vector-paced
            nc.scalar.copy(ot[:], ops[:, ts(c, P)])
        else:
            nc.vector.tensor_copy(ot[:], ops[:, ts(c, P)])
        ptf = tps.tile([P, H + 1], BF16, tag="t", name=f"ft_{tt}")
        nc.tensor.transpose(ptf[:], ot[:], ident_sb[0 : H + 1, 0 : H + 1])
        rch = fin.tile([P, 1], F32, tag="rch", name=f"rch_{tt}")
        nc.vector.reciprocal(rch[:], ptf[:, H : H + 1])
        nc.vector.tensor_scalar_mul(o_sb[:, tt, :], ptf[:, 0:H], rch[:])

    outr = out.rearrange("(t p) h -> p t h", p=P)

    emit_S(1, 0)
    emit_S(1, 1)
    emit_v(0)
    for t in range(0, 4):
        emit_vtrans(t)
    emit_v(1)
    for t in range(4, 8):
        emit_vtrans(t)

    ops0 = op.tile([H + 1, GW], F32, tag="o", name="ops_0")
    for jj in range(8):
        emit_AV(0, jj, ops0)
        if jj >= 1:
            emit_fin(0, jj - 1, ops0)  # delayed one step to keep PE flowing
    emit_fin(0, 7, ops0)
    nc.sync.dma_start(outr[:, 0:8, :], o_sb[:, 0:8, :])

    ops1 = op.tile([H + 1, GW], F32, tag="o", name="ops_1")
    for jj in range(16):
        emit_AV(1, jj, ops1)
        # exp-paced S drip: +3 ahead of its AV. S(1, jj+3) gates on
        # exp(1, jj+1) — exactly what the NEXT iteration's AV waits on, so
        # the in-order PE queue never adds a stall, and each strip reaches
        # ScalarE ~0.3us before the exp stream needs it.
        if jj == 0:
            emit_S(1, 2)
        if 3 <= jj + 3 <= 15:
            emit_S(1, jj + 3)
        if jj == 1:
            emit_v(2)
        elif jj == 2:
            emit_vtrans(8)
            emit_vtrans(9)
        elif jj == 3:
            emit_vtrans(10)
            emit_vtrans(11)
        elif jj == 4:
            emit_v(3)
        elif jj == 5:
            emit_vtrans(12)
            emit_vtrans(13)
        elif jj == 6:
            emit_vtrans(14)
            emit_vtrans(15)
        if jj >= 9:
            emit_fin(1, jj - 9, ops1)
        if jj == 12:
            nc.sync.dma_start(outr[:, 8:12, :], o_sb[:, 8:12, :])
    emit_fin(1, 7, ops1)
    nc.sync.dma_start(outr[:, 12:15, :], o_sb[:, 12:15, :])
    nc.sync.dma_start(outr[:, 15:16, :], o_sb[:, 15:16, :])


def build():
    from contextlib import ExitStack

    nc = bacc.Bacc("TRN2", target_bir_lowering=False, debug=False, num_devices=B)
    xlo = nc.dram_tensor("xlo", [P, NCH, 1024], BF16, kind="ExternalInput").ap()
    xhi = nc.dram_tensor("xhi", [P, NCH, 1024], BF16, kind="ExternalInput").ap()
    wqk = nc.dram_tensor("wqk", [C, P], BF16, kind="ExternalInput").ap()
    wv = nc.dram_tensor("wv", [C, H], BF16, kind="ExternalInput").ap()
    ident = nc.dram_tensor("ident", [P, P], BF16, kind="ExternalInput").ap()
    tri = nc.dram_tensor("tri", [P, P], BF16, kind="ExternalInput").ap()
    out = nc.dram_tensor("o", [T, H], F32, kind="ExternalOutput").ap()
    with tile.TileContext(nc) as tc, ExitStack() as ctx:
        _emit(tc, ctx, xlo, xhi, wqk, wv, ident, tri, out)
    nc.compile()
    return nc


_NC = None


def _get_nc():
    global _NC
    if _NC is None:
        _NC = build()
    return _NC


def make_in_maps(x, Wk, Wq, Wv):
    bf = ml_dtypes.bfloat16
    wqk = np.ascontiguousarray(np.concatenate([Wq, Wk], axis=1)).astype(bf)
    wv = np.ascontiguousarray(np.asarray(Wv)).astype(bf)
    ident = np.eye(P, dtype=np.float32).astype(bf)
    # upper-tri (keep j<=i) 0/1 mask for the diagonal blocks of S^T[j, i]
    tri = np.where(np.arange(P)[:, None] <= np.arange(P)[None, :], 1.0, 0.0).astype(bf)
    maps = []
    for b in range(B):
        xT = np.asarray(x[b]).T.astype(bf)              # [C, T]
        r = xT.reshape(NCH, P, T).transpose(1, 0, 2)    # [P, chunk, T]
        maps.append(
            {
                "xlo": np.ascontiguousarray(r[:, :, 0:1024]),
                "xhi": np.ascontiguousarray(r[:, :, 1024:2048]),
                "wqk": wqk,
                "wv": wv,
                "ident": ident,
                "tri": tri,
            }
        )
    return maps


def kernel(x, Wk, Wq, Wv):
    from concourse.bass_utils import run_bass_kernel_spmd

    nc = _get_nc()
    in_maps = make_in_maps(x, Wk, Wq, Wv)
    r = run_bass_kernel_spmd(nc, in_maps, core_ids=list(range(B)))
    out = np.stack([r.results[b]["o"] for b in range(B)])
    return np.ascontiguousarray(out, dtype=np.float32)
